# revision 1
# baseline (speedup 1.0000x reference)
"""Trainium2 Bass kernel for the AllPairs triplet-index sampling problem.

Problem (from the reference):
  B=1024 embeddings with balanced labels (C=128 classes, S=8 per class).
  Output is the triplet index expansion
    anchor_idx = repeat(pa, NNEG), pos_idx = repeat(pp, NNEG),
    neg_idx    = neg_per_anchor[pa].reshape(-1)
  where (pa, pp) enumerates the NPOS=B*(S-1)=7168 positive pairs in
  row-major order and neg_per_anchor[i] lists the NNEG=1016 ascending
  indices j with labels[j] != labels[i].

Sharding: the positive-pair axis is split into 8 contiguous slabs of 896
pairs = 128 anchors per core (pair k belongs to anchor k//7, so a
contiguous pair slab is a contiguous anchor slab). Each core handles its
128 anchors as the 128 SBUF partitions.

Per-core algorithm (one anchor per partition, int16 compute for the DVE
2x perf mode; every value < 2^11 so int16/f32 are exact):
  eq[p,j]   = labels[j] == labels[anchor_p]
  rank[p,j] = prefix sum of eq (tensor_tensor_scan)
  idx[p,j]  = j - rank + eq*(1024 - j)   -- a bijection on [0,1024):
              non-members land at their negative-rank 0..1015 ascending,
              members at 1024-rank (1016..1023, descending member order)
  scat      = one gpsimd local_scatter of j by idx
  negatives = scat[:, 0:1016], members u = scat[:, 1016:1024]
  pp        = the 7 members != anchor, via a vectorized select on u
The three [128, 7*1016] output slabs are then written HBM-roofline
style, spread over three DMA paths so the write stream never stalls:
anchor (per-partition constant, ready first) on the ACT HWDGE ring with
a x7 broadcast access pattern, negatives via a SWDGE DMA that casts
int16->int32 inline (also x7 broadcast, SBUF holds one copy), and
positives as a contiguous int32 tile on the SP HWDGE ring. Total
per-core write is 3 x 3.64 MB; with all 8 cores saturating chip HBM
this bounds the kernel at ~30us of DMA + ~10us fixed overhead.
"""

import numpy as np

from concourse import bacc, mybir, tile
from concourse.bass_utils import run_bass_kernel_spmd

B = 1024          # batch
C = 128           # classes
S = B // C        # samples per class (8)
PER = S - 1       # positives per anchor (7)
NNEG = B - S      # negatives per anchor (1016)
ACH = 128         # anchors per core
N_CORES = 8

f32 = mybir.dt.float32
i32 = mybir.dt.int32
i16 = mybir.dt.int16
i8 = mybir.dt.int8

_NC = None


def _strip_const_memsets(nc):
    """Drop the four const-tile memsets Bass emits at construction.

    This kernel never reads the const-* tiles (walrus verifies: "memory
    location with no reader"), and they sit on the gpsimd stream right
    before the init barrier, delaying kernel start by ~1us. Only strips
    when exactly the expected four are found; otherwise leaves the graph
    untouched (correctness never depends on the strip).
    """
    try:
        hits = []
        for bb in nc.m.functions[0].blocks:
            for ins in bb.instructions:
                if type(ins).__name__ == "InstMemset":
                    outs = getattr(ins, "outs", []) or []
                    names = [getattr(getattr(getattr(o, "bass_ap", None),
                                             "tensor", None), "name", "")
                             for o in outs]
                    if any(n.startswith("const-") for n in names):
                        hits.append((bb, ins))
        if len(hits) == 4:
            for bb, ins in hits:
                bb.instructions.remove(ins)
    except Exception:
        pass
    # With the const memsets gone there is no cross-engine preamble state
    # left, so the construction-time all_engine_barrier (per-engine drain +
    # barrier_* event semaphores in block 0) only delays the body; every
    # body-level cross-engine dependency is sequenced by Tile's semaphores.
    # Strip it only when the exact expected pattern is present.
    try:
        bb0 = nc.m.functions[0].blocks[0]
        evs = [i for i in bb0.instructions
               if type(i).__name__ == "InstEventSemaphore"
               and str(i.name).startswith("barrier_")]
        drains = [i for i in bb0.instructions if type(i).__name__ == "InstDrain"]
        if len(evs) == 6 and len(drains) == 5:
            for ins in evs + drains:
                bb0.instructions.remove(ins)
    except Exception:
        pass


def _build():
    global _NC
    if _NC is not None:
        return _NC
    nc = bacc.Bacc("TRN2", target_bir_lowering=False, debug=False,
                   num_devices=N_CORES)

    # labels as int8 (values < 128, replicated to all partitions), plus the
    # two int16 iota tables packed together: [:, 0:B] = j, [:, B:2B] = 1024 - j
    lab_in = nc.declare_dram_parameter("lab8", [ACH, B], i8, isOutput=False)
    iotas_in = nc.declare_dram_parameter("iotas16", [ACH, 2 * B], i16, isOutput=False)
    # tiny per-core input: [:, 0] = labels[anchor_p], [:, 1] = anchor id
    tinyf = nc.declare_dram_parameter("tinyf", [ACH, 2], f32, isOutput=False)

    anchor_out = nc.declare_dram_parameter("anchor_out", [ACH, PER, NNEG], i32, isOutput=True)
    pos_out = nc.declare_dram_parameter("pos_out", [ACH, PER, NNEG], i32, isOutput=True)
    neg_out = nc.declare_dram_parameter("neg_out", [ACH, PER, NNEG], i32, isOutput=True)

    op = mybir.AluOpType
    with tile.TileContext(nc) as tc:
        with tc.tile_pool(name="p", bufs=1) as pool:
            t_lab = pool.tile([ACH, B], i8)
            t_iotas = pool.tile([ACH, 2 * B], i16)
            t_tinyf = pool.tile([ACH, 2], f32)
            t_ones = pool.tile([ACH, B], i16)
            t_eq = pool.tile([ACH, B], i16)
            t_rank = pool.tile([ACH, B], i16)
            t_tmpb = pool.tile([ACH, B], i16)   # j - rank
            t_x = pool.tile([ACH, B], i16)      # eq * (1024 - j)
            t_idx = pool.tile([ACH, B], i16)
            t_scat = pool.tile([ACH, B], i16)
            t_anc32 = pool.tile([ACH, NNEG], i32)
            t_uf = pool.tile([ACH, S], f32)
            t_cm = pool.tile([ACH, PER], f32)
            t_dq = pool.tile([ACH, PER], f32)
            t_dq2 = pool.tile([ACH, PER], f32)
            t_ppr = pool.tile([ACH, PER], f32)
            t_pos32 = pool.tile([ACH, PER, NNEG], i32)

            lab16 = t_lab[:, :]
            iota16 = t_iotas[:, 0:B]
            iotar16 = t_iotas[:, B:2 * B]

            # inputs: tiny first (anchor path depends only on it), then
            # labels (gates the whole compute chain), then the iota tables
            nc.scalar.dma_start(t_tinyf[:, :], tinyf[:, :])
            nc.sync.dma_start(t_lab[:, :], lab_in[:, :])
            nc.sync.dma_start(t_iotas[:, :], iotas_in[:, :])

            nc.gpsimd.memset(t_ones[:, :], 1)

            # anchor slab: every element of row p is the global anchor id;
            # int32 tile, fanned out x7 on the ACT HWDGE ring.
            nc.vector.tensor_scalar(t_anc32[:, :], t_ones[:, :NNEG],
                                    0.0, t_tinyf[:, 1:2], op.mult, op.add)
            nc.scalar.dma_start(
                anchor_out[:, :, :],
                t_anc32[:, :].unsqueeze(1).broadcast_to([ACH, PER, NNEG]))

            # eq[p, j] = labels[j] == labels[anchor_p]
            nc.vector.tensor_scalar(t_eq[:, :], lab16,
                                    t_tinyf[:, 0:1], None, op.is_equal)
            # rank[p, j] = inclusive running count of members
            nc.vector.tensor_tensor_scan(t_rank[:, :], t_ones[:, :], t_eq[:, :],
                                         0.0, op.mult, op.add)
            # idx = (j - rank) + eq*(1024 - j): bijection on [0,1024)
            # (pure tensor_tensor ops with all-int16 operands for 2x mode)
            nc.vector.tensor_tensor(t_tmpb[:, :], iota16, t_rank[:, :], op.subtract)
            nc.vector.tensor_tensor(t_x[:, :], t_eq[:, :], iotar16, op.mult)
            nc.vector.tensor_tensor(t_idx[:, :], t_tmpb[:, :], t_x[:, :], op.add)

            nc.gpsimd.local_scatter(t_scat[:, :], iota16, t_idx[:, :],
                                    channels=ACH, num_elems=B, num_idxs=B)

            # negatives: slots 0..1015; SWDGE DMA casts int16->int32, x7 fan-out
            nc.gpsimd.dma_start(
                neg_out[:, :, :],
                t_scat[:, :NNEG].unsqueeze(1).broadcast_to([ACH, PER, NNEG]))

            # members u_k = scat[1016+k] = q_{7-k} (descending).
            # ppRev[s] = u[s+1] if u[s+1] < anchor else u[s]; pp_t = ppRev[6-t].
            nc.vector.tensor_copy(t_uf[:, :], t_scat[:, NNEG:B])
            nc.vector.tensor_scalar(t_cm[:, :], t_uf[:, 1:S],
                                    t_tinyf[:, 1:2], None, op.is_lt)
            nc.vector.tensor_tensor(t_dq[:, :], t_uf[:, 1:S], t_uf[:, 0:PER], op.subtract)
            nc.vector.tensor_tensor(t_dq2[:, :], t_cm[:, :], t_dq[:, :], op.mult)
            nc.vector.tensor_tensor(t_ppr[:, :], t_uf[:, 0:PER], t_dq2[:, :], op.add)
            for t in range(PER):
                nc.vector.tensor_scalar(t_pos32[:, t, :], t_iotas[:, 0:NNEG],
                                        0.0, t_ppr[:, PER - 1 - t:PER - t], op.mult, op.add)
            # contiguous DMA on the sync HWDGE ring, parallel to the SWDGE neg DMA
            nc.sync.dma_start(pos_out[:, :, :], t_pos32[:, :, :])
    _strip_const_memsets(nc)
    nc.compile()
    _NC = nc
    return nc


def _in_maps(labels):
    lab = np.asarray(labels).astype(np.int16)
    lab_rep = np.ascontiguousarray(np.broadcast_to(lab.astype(np.int8)[None, :], (ACH, B)))
    iotas = np.empty((ACH, 2 * B), dtype=np.int16)
    iotas[:, 0:B] = np.arange(B, dtype=np.int16)[None, :]
    iotas[:, B:2 * B] = B - np.arange(B, dtype=np.int16)[None, :]
    maps = []
    for d in range(N_CORES):
        sl = slice(d * ACH, (d + 1) * ACH)
        tf = np.empty((ACH, 2), dtype=np.float32)
        tf[:, 0] = lab[sl].astype(np.float32)
        tf[:, 1] = np.arange(d * ACH, (d + 1) * ACH, dtype=np.float32)
        maps.append({"lab8": lab_rep, "iotas16": iotas, "tinyf": tf})
    return maps


def _gather(results):
    anchor = np.concatenate([results[d]["anchor_out"].reshape(-1)
                             for d in range(N_CORES)]).astype(np.int32, copy=False)
    pos = np.concatenate([results[d]["pos_out"].reshape(-1)
                          for d in range(N_CORES)]).astype(np.int32, copy=False)
    neg = np.concatenate([results[d]["neg_out"].reshape(-1)
                          for d in range(N_CORES)]).astype(np.int32, copy=False)
    return anchor, pos, neg


def run(labels, trace=False):
    nc = _build()
    res = run_bass_kernel_spmd(nc, _in_maps(labels),
                               core_ids=list(range(N_CORES)), trace=trace)
    return _gather(res.results), res


def kernel(embeddings=None, labels=None, **_):
    (anchor, pos, neg), _res = run(labels, trace=False)
    return anchor, pos, neg



# revision 2
# speedup vs baseline: 1.2555x; 1.2555x over previous
"""Trainium2 Bass kernel for the AllPairs triplet-index sampling problem.

Problem (from the reference):
  B=1024 embeddings with balanced labels (C=128 classes, S=8 per class).
  Output is the triplet index expansion
    anchor_idx = repeat(pa, NNEG), pos_idx = repeat(pp, NNEG),
    neg_idx    = neg_per_anchor[pa].reshape(-1)
  where (pa, pp) enumerates the NPOS=B*(S-1)=7168 positive pairs in
  row-major order and neg_per_anchor[i] lists the NNEG=1016 ascending
  indices j with labels[j] != labels[i].

Sharding: the positive-pair axis is split into 8 contiguous slabs of 896
pairs = 128 anchors per core (pair k belongs to anchor k//7, so a
contiguous pair slab is a contiguous anchor slab). Each core handles its
128 anchors as the 128 SBUF partitions.

Per-core algorithm (one anchor per partition; int16 compute for the DVE
packed perf modes; every value < 2^11 so int16/f32 are exact):
  eq[p,j]  = labels[j] == labels[anchor_p]
  eqb[p,j] = 1 - eq  (not_equal)
  nn[p,j]  = prefix sum of eqb with initial=-1  (tensor_tensor_scan)
             = (# non-members <= j) - 1
  x[p,j]   = eq * (1024 - j)
  idx      = nn + x   -- a bijection on [0,1024): non-members land at
             their negative-rank 0..1015 ascending, members at
             1024-rank (1016..1023, descending member order)
  scat     = one gpsimd local_scatter of j by idx
  negatives = scat[:, 0:1016], members u = scat[:, 1016:1024]
  pp       = the 7 members != anchor, via a vectorized select on u

All three output slabs are written as int16 (every index < 1024; the
host-side gather widens to int32, value-preserving) which halves the
HBM write traffic to 3 x 1.82 MB per core -- the binding roofline is
the ~358 GB/s per-core HBM write limit. The end time is governed by
(scatter completion) + (neg+pos bytes)/rate, so the anchor slab (ready
immediately) is written first on the ACT HWDGE ring while the compute
chain runs; negatives go on the SP HWDGE ring right after the scatter
(x7 broadcast read of one SBUF copy), and positives follow on the ACT
ring in 3 chunks as the DVE finishes each group of rows.
"""

import numpy as np

from concourse import bacc, mybir, tile
from concourse.bass_utils import run_bass_kernel_spmd

B = 1024          # batch
C = 128           # classes
S = B // C        # samples per class (8)
PER = S - 1       # positives per anchor (7)
NNEG = B - S      # negatives per anchor (1016)
ACH = 128         # anchors per core
N_CORES = 8

f32 = mybir.dt.float32
i32 = mybir.dt.int32
i16 = mybir.dt.int16

_NC = None


def _strip_const_memsets(nc):
    """Drop the four const-tile memsets Bass emits at construction.

    This kernel never reads the const-* tiles (walrus verifies: "memory
    location with no reader"), and they sit on the gpsimd stream right
    before the init barrier, delaying kernel start by ~1us. Only strips
    when exactly the expected four are found; otherwise leaves the graph
    untouched (correctness never depends on the strip).
    """
    try:
        hits = []
        for bb in nc.m.functions[0].blocks:
            for ins in bb.instructions:
                if type(ins).__name__ == "InstMemset":
                    outs = getattr(ins, "outs", []) or []
                    names = [getattr(getattr(getattr(o, "bass_ap", None),
                                             "tensor", None), "name", "")
                             for o in outs]
                    if any(n.startswith("const-") for n in names):
                        hits.append((bb, ins))
        if len(hits) == 4:
            for bb, ins in hits:
                bb.instructions.remove(ins)
    except Exception:
        pass
    # With the const memsets gone there is no cross-engine preamble state
    # left, so the construction-time all_engine_barrier (per-engine drain +
    # barrier_* event semaphores in block 0) only delays the body; every
    # body-level cross-engine dependency is sequenced by Tile's semaphores.
    # Strip it only when the exact expected pattern is present.
    try:
        bb0 = nc.m.functions[0].blocks[0]
        evs = [i for i in bb0.instructions
               if type(i).__name__ == "InstEventSemaphore"
               and str(i.name).startswith("barrier_")]
        drains = [i for i in bb0.instructions if type(i).__name__ == "InstDrain"]
        if len(evs) == 6 and len(drains) == 5:
            for ins in evs + drains:
                bb0.instructions.remove(ins)
    except Exception:
        pass


def _build():
    global _NC
    if _NC is not None:
        return _NC
    nc = bacc.Bacc("TRN2", target_bir_lowering=False, debug=False,
                   num_devices=N_CORES)

    # labels as int16 (replicated to all partitions) so the DVE compare ops
    # run in the packed 16-bit perf modes; iota tables [:, 0:B] = j,
    # [:, B:2B] = 1024 - j
    lab_in = nc.declare_dram_parameter("lab16", [ACH, B], i16, isOutput=False)
    iotas_in = nc.declare_dram_parameter("iotas16", [ACH, 2 * B], i16, isOutput=False)
    # tiny per-core input: [:, 0] = labels[anchor_p], [:, 1] = anchor id
    tinyf = nc.declare_dram_parameter("tinyf", [ACH, 2], f32, isOutput=False)

    anchor_out = nc.declare_dram_parameter("anchor_out", [ACH, PER, NNEG], i16, isOutput=True)
    pos_out = nc.declare_dram_parameter("pos_out", [ACH, PER, NNEG], i16, isOutput=True)
    neg_out = nc.declare_dram_parameter("neg_out", [ACH, PER, NNEG], i16, isOutput=True)

    op = mybir.AluOpType
    with tile.TileContext(nc) as tc:
        with tc.tile_pool(name="p", bufs=1) as pool:
            t_lab = pool.tile([ACH, B], i16)
            t_iotas = pool.tile([ACH, 2 * B], i16)
            t_tinyf = pool.tile([ACH, 2], f32)
            t_ones = pool.tile([ACH, B], i16)
            t_eq = pool.tile([ACH, B], i16)
            t_eqb = pool.tile([ACH, B], i16)
            t_nn = pool.tile([ACH, B], i16)
            t_x = pool.tile([ACH, B], i16)      # eq * (1024 - j)
            t_idx = pool.tile([ACH, B], i16)
            t_scat = pool.tile([ACH, B], i16)
            t_anc16 = pool.tile([ACH, NNEG], i16)
            t_uf = pool.tile([ACH, S], f32)
            t_cm = pool.tile([ACH, PER], f32)
            t_dq = pool.tile([ACH, PER], f32)
            t_dq2 = pool.tile([ACH, PER], f32)
            t_ppr = pool.tile([ACH, PER], f32)
            t_pos16 = pool.tile([ACH, PER, NNEG], i16)

            iota16 = t_iotas[:, 0:B]
            iotar16 = t_iotas[:, B:2 * B]

            # inputs: tiny first on the ACT ring (anchor path depends only
            # on it), labels on the SP ring (gates the compute chain), then
            # the iota tables (not needed until after the scan)
            nc.scalar.dma_start(t_tinyf[:, :], tinyf[:, :])
            nc.sync.dma_start(t_lab[:, :], lab_in[:, :])
            nc.sync.dma_start(t_iotas[:, :], iotas_in[:, :])

            nc.gpsimd.memset(t_ones[:, :], 1)

            # anchor slab: every element of row p is the global anchor id;
            # int16 tile, fanned out x7 on the ACT HWDGE ring immediately.
            nc.vector.tensor_scalar(t_anc16[:, :], t_ones[:, :NNEG],
                                    0.0, t_tinyf[:, 1:2], op.mult, op.add)
            nc.scalar.dma_start(
                anchor_out[:, :, :],
                t_anc16[:, :].unsqueeze(1).broadcast_to([ACH, PER, NNEG]))

            # eq / eqb from labels (both 16-bit single-src ops -> 4x mode)
            nc.vector.tensor_scalar(t_eq[:, :], t_lab[:, :],
                                    t_tinyf[:, 0:1], None, op.is_equal)
            nc.vector.tensor_scalar(t_eqb[:, :], t_lab[:, :],
                                    t_tinyf[:, 0:1], None, op.not_equal)
            # nn[p, j] = (# non-members <= j) - 1 via scan with initial=-1
            nc.vector.tensor_tensor_scan(t_nn[:, :], t_ones[:, :], t_eqb[:, :],
                                         -1.0, op.mult, op.add)
            # x = eq*(1024 - j); idx = nn + x: bijection on [0,1024)
            nc.vector.tensor_tensor(t_x[:, :], t_eq[:, :], iotar16, op.mult)
            nc.vector.tensor_tensor(t_idx[:, :], t_nn[:, :], t_x[:, :], op.add)

            nc.gpsimd.local_scatter(t_scat[:, :], iota16, t_idx[:, :],
                                    channels=ACH, num_elems=B, num_idxs=B)

            # negatives: slots 0..1015, x7 broadcast on the SP HWDGE ring
            nc.sync.dma_start(
                neg_out[:, :, :],
                t_scat[:, :NNEG].unsqueeze(1).broadcast_to([ACH, PER, NNEG]))

            # members u_k = scat[1016+k] = m_{7-k} (descending).
            # ppRev[s] = u[s+1] if u[s+1] < anchor else u[s]; pp_t = ppRev[6-t].
            nc.vector.tensor_copy(t_uf[:, :], t_scat[:, NNEG:B])
            nc.vector.tensor_scalar(t_cm[:, :], t_uf[:, 1:S],
                                    t_tinyf[:, 1:2], None, op.is_lt)
            nc.vector.tensor_tensor(t_dq[:, :], t_uf[:, 1:S], t_uf[:, 0:PER], op.subtract)
            nc.vector.tensor_tensor(t_dq2[:, :], t_cm[:, :], t_dq[:, :], op.mult)
            nc.vector.tensor_tensor(t_ppr[:, :], t_uf[:, 0:PER], t_dq2[:, :], op.add)
            # pos rows (int16 single-src -> 4x), DMA'd in 3 chunks on the
            # ACT ring so the write stream starts as soon as rows exist
            for t in range(PER):
                nc.vector.tensor_scalar(t_pos16[:, t, :], t_iotas[:, 0:NNEG],
                                        0.0, t_ppr[:, PER - 1 - t:PER - t], op.mult, op.add)
                if t == 1:
                    nc.scalar.dma_start(pos_out[:, 0:2, :], t_pos16[:, 0:2, :])
                elif t == 4:
                    nc.scalar.dma_start(pos_out[:, 2:5, :], t_pos16[:, 2:5, :])
                elif t == PER - 1:
                    nc.scalar.dma_start(pos_out[:, 5:PER, :], t_pos16[:, 5:PER, :])
    _strip_const_memsets(nc)
    nc.compile()
    _NC = nc
    return nc


def _in_maps(labels):
    lab = np.asarray(labels).astype(np.int16)
    lab_rep = np.ascontiguousarray(np.broadcast_to(lab[None, :], (ACH, B)))
    iotas = np.empty((ACH, 2 * B), dtype=np.int16)
    iotas[:, 0:B] = np.arange(B, dtype=np.int16)[None, :]
    iotas[:, B:2 * B] = B - np.arange(B, dtype=np.int16)[None, :]
    maps = []
    for d in range(N_CORES):
        sl = slice(d * ACH, (d + 1) * ACH)
        tf = np.empty((ACH, 2), dtype=np.float32)
        tf[:, 0] = lab[sl].astype(np.float32)
        tf[:, 1] = np.arange(d * ACH, (d + 1) * ACH, dtype=np.float32)
        maps.append({"lab16": lab_rep, "iotas16": iotas, "tinyf": tf})
    return maps


def _gather(results):
    anchor = np.concatenate([results[d]["anchor_out"].reshape(-1)
                             for d in range(N_CORES)]).astype(np.int32)
    pos = np.concatenate([results[d]["pos_out"].reshape(-1)
                          for d in range(N_CORES)]).astype(np.int32)
    neg = np.concatenate([results[d]["neg_out"].reshape(-1)
                          for d in range(N_CORES)]).astype(np.int32)
    return anchor, pos, neg


def run(labels, trace=False):
    nc = _build()
    res = run_bass_kernel_spmd(nc, _in_maps(labels),
                               core_ids=list(range(N_CORES)), trace=trace)
    return _gather(res.results), res


def kernel(embeddings=None, labels=None, **_):
    (anchor, pos, neg), _res = run(labels, trace=False)
    return anchor, pos, neg
